# revision 1
# baseline (speedup 1.0000x reference)
"""Trainium2 Bass kernel for ContextualLoss3D over 8x8x8 patches.

Full inputs x, y: (2, 32, 48, 48, 48) f32. Output: scalar f32 loss.

Strategy: shard the 216 patches across 8 NeuronCores (27 patches each, both
batch elements of a patch on the same core so the y-mean needs no collective).
Each core processes its 54 (n, p) pairs:
  - pack 4 pairs (2 patches x 2 batch) into the 128 SBUF partitions
  - y_mu per (patch, channel) via a free-dim row-sum + one PE matmul that
    combines the two batch halves and broadcasts back to all 4 slots
  - center x/y, channel-norms via squares + block-diagonal-ones PE matmul,
    1/sqrt via ACT log/exp (same table set as the main exp)
  - per pair: gram G = xc^T @ yn on PE (4x (128,512) PSUM chunks),
    row-max on DVE, then w = exp(scale_i*G + bias_i) on ACT with fused
    row-sum accumulation; cx column-max via scaled max-accumulate
    (DVE/GPSIMD), PE transposes, and a free-dim reduce
  - per-core output: sums over its patches of per-patch column-max (2,128,8)
Host: gathers the 8 partial sums, finishes mean over patches, -log, mean.
"""

import numpy as np

import concourse.bass as bass
import concourse.tile as tile
from concourse import mybir
from concourse.bass_utils import run_bass_kernel_spmd

PATCH = 8
N_BATCH = 2
C = 32
M = 512  # 8^3 positions per patch
P_TOT = 216  # (48/8)^3 patches
NCORES = 8
PPC = P_TOT // NCORES  # 27 patches per core
NGROUP = (PPC + 1) // 2  # 14 groups of (2 patches x 2 batch) = 4 pairs
EPS = 1e-5

f32 = mybir.dt.float32
f16 = mybir.dt.float16
AX = mybir.AxisListType.X
OP = mybir.AluOpType
AF = mybir.ActivationFunctionType

_BUILT = None  # cached (nc,) module


def _split_multiwaits(nc):
    """This walrus build supports ONE sync wait per TPB instruction (the 64B
    ISA word has a single events slot). Tile can emit several; split the
    extras into standalone EventSemaphore waits on the same engine, placed
    immediately before the instruction (same sequencer => same semantics)."""
    n_new = 0
    for fn in nc.m.functions:
        for bb in fn.blocks:
            out = []
            for inst in bb.instructions:
                si = inst.sync_info
                if si is not None and si.on_wait and len(si.on_wait) > 1:
                    waits = list(si.on_wait)
                    for w in waits[:-1]:
                        ev = mybir.InstEventSemaphore(
                            name=f"{inst.name}-w{n_new}", ins=[], outs=[]
                        )
                        ev.engine = inst.engine
                        ev.sync_info = mybir.SyncInfo(on_wait=[w], on_update=[])
                        out.append(ev)
                        n_new += 1
                    inst.sync_info = mybir.SyncInfo(
                        on_wait=[waits[-1]], on_update=list(si.on_update)
                    )
                out.append(inst)
            bb.instructions = out
    return n_new


def _pairs_in_group(g):
    # last group has only 1 real patch (27 = 13*2 + 1): pairs q=0 (n=0), q=1 (n=1)
    return 4 if g < NGROUP - 1 else 2


def _build_module():
    nc = bass.Bass(
        "TRN2",
        debug=False,
        enable_asserts=False,
        target_bir_lowering=False,
        num_devices=NCORES,
    )

    X = nc.dram_tensor("xs", [NGROUP, 128, M], f32, kind="ExternalInput").ap()
    Y = nc.dram_tensor("ys", [NGROUP, 128, M], f32, kind="ExternalInput").ap()
    WMU = nc.dram_tensor("wmu", [128, 128], f32, kind="ExternalInput").ap()
    BD = nc.dram_tensor("bd", [128, 4], f32, kind="ExternalInput").ap()
    BDT = nc.dram_tensor("bdt", [4, 128], f32, kind="ExternalInput").ap()
    ID4 = nc.dram_tensor("id4", [4, 4], f32, kind="ExternalInput").ap()
    ID128 = nc.dram_tensor("id128", [128, 128], f16, kind="ExternalInput").ap()
    OUT = nc.dram_tensor("acc_out", [N_BATCH, 128, 8], f32, kind="ExternalOutput").ap()

    with tile.TileContext(nc) as tc:
        with (
            tc.tile_pool(name="consts", bufs=1) as consts,
            tc.tile_pool(name="io", bufs=3) as io,
            tc.tile_pool(name="sb", bufs=3) as sb,
            tc.tile_pool(name="tiny", bufs=6) as tiny,
            tc.tile_pool(name="wpool", bufs=3) as wpool,
            tc.tile_pool(name="accp", bufs=1) as accp,
            tc.tile_pool(name="psA", bufs=2, space="PSUM") as psA,
            tc.tile_pool(name="psB", bufs=4, space="PSUM") as psB,
        ):
            wmu_sb = consts.tile([128, 128], f32, tag="wmu")
            nc.sync.dma_start(wmu_sb, WMU)
            bd_sb = consts.tile([128, 4], f32, tag="bd")
            nc.sync.dma_start(bd_sb, BD)
            bdt_sb = consts.tile([4, 128], f32, tag="bdt")
            nc.sync.dma_start(bdt_sb, BDT)
            id4_sb = consts.tile([4, 4], f32, tag="id4")
            nc.sync.dma_start(id4_sb, ID4)
            id128_sb = consts.tile([128, 128], f16, tag="id128")
            nc.sync.dma_start(id128_sb, ID128)
            c24 = consts.tile([128, 1], f32, tag="c24")
            nc.vector.memset(c24, 1e-24)

            acc = [
                accp.tile([128, 8], f32, tag=f"acc{n}", name=f"acc{n}")
                for n in range(N_BATCH)
            ]
            for a in acc:
                nc.vector.memset(a, 0.0)

            for g in range(NGROUP):
                npair = _pairs_in_group(g)

                xg = io.tile([128, M], f32, tag="xg")
                nc.sync.dma_start(xg, X[g])
                yg = io.tile([128, M], f32, tag="yg")
                nc.sync.dma_start(yg, Y[g])

                # ---- group prep: mean, centering, channel norms ----
                ysum = tiny.tile([128, 1], f32, tag="ysum")
                nc.vector.reduce_sum(ysum, yg, axis=AX)
                mu_ps = psB.tile([128, 1], f32, tag="psB")
                nc.tensor.matmul(mu_ps, wmu_sb, ysum)
                mu = tiny.tile([128, 1], f32, tag="mu")
                nc.vector.tensor_copy(mu, mu_ps)

                xc = sb.tile([128, M], f32, tag="xc")
                nc.vector.tensor_scalar(xc, xg, mu, None, op0=OP.subtract)
                yc = sb.tile([128, M], f32, tag="yc")
                nc.vector.tensor_scalar(yc, yg, mu, None, op0=OP.subtract)

                xsq = sb.tile([128, M], f32, tag="xsq")
                nc.gpsimd.tensor_mul(xsq, xc, xc)
                ysq = sb.tile([128, M], f32, tag="ysq")
                nc.gpsimd.tensor_mul(ysq, yc, yc)

                sx_ps = psB.tile([4, M], f32, tag="psB")
                nc.tensor.matmul(sx_ps, bd_sb, xsq)
                sy_ps = psB.tile([4, M], f32, tag="psB")
                nc.tensor.matmul(sy_ps, bd_sb, ysq)

                # rinv = (S + 1e-24)^-0.5 via log/exp (same ACT table set as Exp)
                ls = sb.tile([4, 2, M], f32, tag="ls")
                nc.scalar.activation(ls[:, 0, :], sx_ps, AF.Ln, bias=c24[:4])
                nc.scalar.activation(ls[:, 1, :], sy_ps, AF.Ln, bias=c24[:4])
                rinv = sb.tile([4, 2, M], f32, tag="rinv")
                nc.scalar.activation(rinv, ls, AF.Exp, scale=-0.5)

                # broadcast y-norms to all 4 slot blocks; yn = yc * rinv_y
                rny_ps = psB.tile([128, M], f32, tag="psB")
                nc.tensor.matmul(rny_ps, bdt_sb, rinv[:, 1, :])
                yn = sb.tile([128, M], f32, tag="yn")
                nc.vector.tensor_mul(yn, yc, rny_ps)

                # x-norms transposed to per-partition layout: invxT[i', c, q]
                invxT_ps = psB.tile([128, 4, 4], f32, tag="psB")
                for c in range(4):
                    nc.tensor.transpose(
                        invxT_ps[:, c, :], rinv[:, 0, 128 * c : 128 * (c + 1)], id4_sb
                    )
                invxT = tiny.tile([128, 4, 4], f32, tag="invxT")
                nc.vector.tensor_copy(invxT, invxT_ps)

                colmax = tiny.tile([128, 4, 4], f32, tag="colmax")

                # ---- per (n, p) pair ----
                for q in range(npair):
                    lo = 32 * q
                    tp = (lo, 0) if lo else None

                    w = wpool.tile([128, 4, M], f16, tag="w")
                    mx4 = tiny.tile([128, 4], f32, tag="mx4")
                    rowsum = tiny.tile([128, 4], f32, tag="rowsum")
                    ghalves = []
                    for h in range(2):
                        gh = psA.tile([128, 2, M], f32, tag="G")
                        ghalves.append(gh)
                        for cc in range(2):
                            c = 2 * h + cc
                            nc.tensor.matmul(
                                gh[:, cc, :],
                                xc[lo : lo + 32, 128 * c : 128 * (c + 1)],
                                yn[lo : lo + 32, :],
                                tile_position=tp,
                            )
                        nc.vector.reduce_max(mx4[:, 2 * h : 2 * h + 2], gh, axis=AX)

                    # scale_i = invx/d, bias_i = 1 - 1/d, d = 1 + eps - invx*mx
                    cm4 = tiny.tile([128, 4], f32, tag="cm4")
                    nc.vector.tensor_mul(cm4, mx4, invxT[:, :, q])
                    d4 = tiny.tile([128, 4], f32, tag="d4")
                    nc.vector.tensor_scalar(
                        d4, cm4, -1.0, 1.0 + EPS, op0=OP.mult, op1=OP.add
                    )
                    q4 = tiny.tile([128, 4], f32, tag="q4")
                    nc.vector.reciprocal(q4, d4)
                    scale4 = tiny.tile([128, 4], f32, tag="scale4")
                    nc.vector.tensor_mul(scale4, q4, invxT[:, :, q])
                    bias4 = tiny.tile([128, 4], f32, tag="bias4")
                    nc.vector.tensor_scalar(
                        bias4, q4, -1.0, 1.0, op0=OP.mult, op1=OP.add
                    )

                    for h in range(2):
                        for cc in range(2):
                            c = 2 * h + cc
                            nc.scalar.activation(
                                w[:, c, :],
                                ghalves[h][:, cc, :],
                                AF.Exp,
                                bias=bias4[:, c : c + 1],
                                scale=scale4[:, c : c + 1],
                                accum_out=rowsum[:, c : c + 1],
                            )

                    r4 = tiny.tile([128, 4], f32, tag="r4")
                    nc.vector.reciprocal(r4, rowsum)

                    # column-max accumulate: macc = max_c (w_c * r_c).
                    # Pool has no max ALU op in this walrus, so DVE does the
                    # fused scale+max (STT); each op is fp16 2x mode.
                    macc = wpool.tile([128, M], f16, tag="macc")
                    nc.vector.tensor_scalar(
                        macc, w[:, 0, :], r4[:, 0:1], None, op0=OP.mult
                    )
                    for c in range(1, 4):
                        nc.vector.scalar_tensor_tensor(
                            macc, w[:, c, :], r4[:, c : c + 1], macc,
                            op0=OP.mult, op1=OP.max,
                        )

                    t_ps = psB.tile([128, 4, 128], f16, tag="psB")
                    for t in range(4):
                        nc.tensor.transpose(
                            t_ps[:, t, :], macc[:, 128 * t : 128 * (t + 1)], id128_sb
                        )
                    nc.vector.reduce_max(colmax[:, q, :], t_ps, axis=AX)

                # accumulate per-batch: pair q has n = q % 2, sub-slot q // 2
                if npair == 4:
                    for n in range(2):
                        nc.vector.tensor_add(
                            acc[n].rearrange("p (s t) -> p s t", s=2),
                            acc[n].rearrange("p (s t) -> p s t", s=2),
                            colmax[:, n::2, :],
                        )
                else:
                    for n in range(2):
                        nc.vector.tensor_add(
                            acc[n][:, 0:4], acc[n][:, 0:4], colmax[:, n, :]
                        )

            for n in range(N_BATCH):
                nc.sync.dma_start(OUT[n], acc[n])

    _split_multiwaits(nc)
    return nc


def _to_patches(v):
    n, c, h, w, d = v.shape
    p = PATCH
    v = v.reshape(n, c, h // p, p, w // p, p, d // p, p)
    v = v.transpose(0, 2, 4, 6, 1, 3, 5, 7)
    return np.ascontiguousarray(v.reshape(n, -1, c, p**3))


def _pack_core(vp, k):
    # vp: (2, 216, 32, 512) -> (NGROUP, 128, 512) for core k
    sl = vp[:, PPC * k : PPC * (k + 1)]  # (2, 27, 32, 512)
    pad = np.zeros((N_BATCH, 2 * NGROUP - PPC, C, M), np.float32)
    arr = np.concatenate([sl, pad], axis=1)  # (2, 28, 32, 512)
    arr = arr.reshape(N_BATCH, NGROUP, 2, C, M)  # [n, g, two, c, m]
    arr = arr.transpose(1, 2, 0, 3, 4)  # [g, two, n, c, m]
    return np.ascontiguousarray(arr.reshape(NGROUP, 128, M))


def _consts():
    kk, pp = np.meshgrid(np.arange(128), np.arange(128), indexing="ij")
    wmu = np.where(
        (kk % 32 == pp % 32) & (kk // 64 == pp // 64), 1.0 / (N_BATCH * M), 0.0
    ).astype(np.float32)
    bd = np.zeros((128, 4), np.float32)
    bd[np.arange(128), np.arange(128) // 32] = 1.0
    bdt = np.ascontiguousarray(bd.T)
    id4 = np.eye(4, dtype=np.float32)
    id128 = np.eye(128, dtype=np.float16)
    return dict(wmu=wmu, bd=bd, bdt=bdt, id4=id4, id128=id128)


def kernel(x, y):
    global _BUILT
    x = np.ascontiguousarray(np.asarray(x), dtype=np.float32)
    y = np.ascontiguousarray(np.asarray(y), dtype=np.float32)
    xp = _to_patches(x)
    yp = _to_patches(y)

    if _BUILT is None:
        _BUILT = _build_module()
    nc = _BUILT

    consts = _consts()
    in_maps = [
        dict(xs=_pack_core(xp, k), ys=_pack_core(yp, k), **consts)
        for k in range(NCORES)
    ]
    res = run_bass_kernel_spmd(nc, in_maps, core_ids=list(range(NCORES)))

    tot = np.zeros((N_BATCH, 128, 8), np.float64)
    for r in res.results:
        tot += r["acc_out"].astype(np.float64)
    tot4 = tot.reshape(N_BATCH, 128, 2, 4).sum(axis=2)  # (2, 128, 4) [n, j', t]
    cx_tot = tot4.transpose(0, 2, 1).reshape(N_BATCH, M) / P_TOT  # j = 128*t + j'
    loss = np.mean(-np.log(cx_tot + EPS))
    return np.float32(loss)

